# revision 2
# baseline (speedup 1.0000x reference)
import sys
from contextlib import ExitStack

sys.path.insert(0, "/opt/trn_rl_repo")

import numpy as np
import ml_dtypes

import concourse.bass as bass
import concourse.bacc as bacc
import concourse.mybir as mybir
import concourse.tile as tile
from concourse.bass_utils import run_bass_kernel_spmd
from concourse.masks import make_identity

B, N, D, H, HD = 4, 4096, 1024, 16, 64
NCORES = 8
T = (B * N) // NCORES  # 2048 tokens per core
P = 128
NT = T // P            # 16 token tiles per core
KT = D // P            # 8 contraction tiles
E3 = 3 * D

_CACHE = {}


def _name(t):
    return t.name if hasattr(t, "name") else t.tensor.name


def _build():
    bf = mybir.dt.bfloat16
    f32 = mybir.dt.float32
    X = mybir.AxisListType.X
    nc = bacc.Bacc(None, target_bir_lowering=False)
    names = {}
    with tile.TileContext(nc) as tc:
        with ExitStack() as ctx:
            dram = ctx.enter_context(tc.tile_pool(name="dram", bufs=1, space="DRAM"))
            xT_d = dram.tile([D, T], bf, kind="ExternalInput")
            wq_d = dram.tile([D, E3], bf, kind="ExternalInput")
            wo_d = dram.tile([D, D], bf, kind="ExternalInput")
            out_d = dram.tile([T, D], f32, kind="ExternalOutput")
            names["xT"] = _name(xT_d)
            names["wqkvT"] = _name(wq_d)
            names["woT"] = _name(wo_d)
            names["out"] = _name(out_d)

            consts = ctx.enter_context(tc.tile_pool(name="consts", bufs=1))
            xT_sb = consts.tile([P, KT, T], bf)
            wq_sb = consts.tile([P, KT, E3], bf)
            wo_sb = consts.tile([P, KT, D], bf)
            ident = consts.tile([P, P], bf)
            make_identity(nc, ident)
            nc.sync.dma_start(out=xT_sb[:], in_=xT_d[:].rearrange("(k p) t -> p k t", p=P))
            nc.sync.dma_start(out=wq_sb[:], in_=wq_d[:].rearrange("(k p) e -> p k e", p=P))
            nc.sync.dma_start(out=wo_sb[:], in_=wo_d[:].rearrange("(k p) e -> p k e", p=P))

            pool = ctx.enter_context(tc.tile_pool(name="work", bufs=2))
            psum1 = ctx.enter_context(tc.tile_pool(name="psum1", bufs=2, space="PSUM"))
            psum2 = ctx.enter_context(tc.tile_pool(name="psum2", bufs=2, space="PSUM"))
            psum3 = ctx.enter_context(tc.tile_pool(name="psum3", bufs=2, space="PSUM"))

            for i in range(NT):
                tsl = bass.ts(i, P)
                # ---- QKV projection: qkv[t, e] for this 128-token tile ----
                qkv = pool.tile([P, E3], bf, tag="qkv")
                for nch in range(E3 // 512):
                    ps = psum1.tile([P, 512], f32, tag="mm1")
                    for k in range(KT):
                        nc.tensor.matmul(
                            ps[:],
                            xT_sb[:, k, tsl],
                            wq_sb[:, k, bass.ts(nch, 512)],
                            start=(k == 0),
                            stop=(k == KT - 1),
                        )
                    nc.scalar.copy(qkv[:, bass.ts(nch, 512)], ps[:])

                # ---- scores[t, h, g] = sum_d q[t,h,d] k[t,g,d] ----
                qv = qkv[:, 0:D].rearrange("p (h d) -> p h d", d=HD)
                scores = pool.tile([P, H, H], f32, tag="sc")
                tmp = pool.tile([P, H, HD], bf, tag="tmp")
                for g in range(H):
                    kg = qkv[:, D + g * HD : D + (g + 1) * HD]
                    kgb = kg[:, None, :].broadcast_to((P, H, HD))
                    nc.vector.tensor_mul(tmp[:], qv, kgb)
                    nc.vector.reduce_sum(scores[:, :, g : g + 1], tmp[:], axis=X)

                # ---- softmax over g (no max-subtract; |s/32| is small) ----
                we = pool.tile([P, H, H], bf, tag="we")
                den = pool.tile([P, H], f32, tag="den")
                rec = pool.tile([P, H], f32, tag="rec")
                nc.scalar.activation(
                    we[:], scores[:], mybir.ActivationFunctionType.Exp, scale=1.0 / 32.0
                )
                nc.vector.reduce_sum(den[:], we[:], axis=X)
                nc.vector.reciprocal(rec[:], den[:])

                # ---- attn[t, h, d] = sum_g w[t,h,g] v[t,g,d] (unnormalized) ----
                attn = pool.tile([P, H, HD], f32, tag="attn")
                for g in range(H):
                    vg = qkv[:, 2 * D + g * HD : 2 * D + (g + 1) * HD]
                    vgb = vg[:, None, :].broadcast_to((P, H, HD))
                    wgb = we[:, :, g : g + 1].broadcast_to((P, H, HD))
                    if g == 0:
                        nc.vector.tensor_mul(attn[:], wgb, vgb)
                    else:
                        nc.vector.tensor_mul(tmp[:], wgb, vgb)
                        nc.vector.tensor_add(attn[:], attn[:], tmp[:])

                # ---- normalize rows by 1/den per (t, h), cast to bf16 ----
                attnb = pool.tile([P, H, HD], bf, tag="attnb")
                for h in range(H):
                    nc.vector.tensor_scalar_mul(
                        attnb[:, h, :], attn[:, h, :], rec[:, h : h + 1]
                    )

                # ---- transpose attn tile -> [e, t] blocks for output proj ----
                attnb_flat = attnb.rearrange("p h d -> p (h d)")
                attnT = pool.tile([P, KT, P], bf, tag="attnT")
                for c in range(KT):
                    pt = psum2.tile([P, P], bf, tag="pt")
                    nc.tensor.transpose(pt[:], attnb_flat[:, bass.ts(c, P)], ident[:])
                    nc.scalar.copy(attnT[:, c, :], pt[:])

                # ---- output projection ----
                outt = pool.tile([P, D], f32, tag="outt")
                for nch in range(D // 512):
                    po = psum3.tile([P, 512], f32, tag="po")
                    for k in range(KT):
                        nc.tensor.matmul(
                            po[:],
                            attnT[:, k, :],
                            wo_sb[:, k, bass.ts(nch, 512)],
                            start=(k == 0),
                            stop=(k == KT - 1),
                        )
                    nc.scalar.copy(outt[:, bass.ts(nch, 512)], po[:])
                nc.sync.dma_start(out=out_d[tsl, :], in_=outt[:])
    nc.compile()
    return nc, names


def make_in_maps(inputs):
    if "nc" not in _CACHE:
        _CACHE["nc"], _CACHE["names"] = _build()
    names = _CACHE["names"]
    x, Wqkv, Wo = inputs["x"], inputs["Wqkv"], inputs["Wo"]
    bf = ml_dtypes.bfloat16
    xt = np.ascontiguousarray(
        np.asarray(x, dtype=np.float32).reshape(B * N, D).T
    )  # [D, B*N]
    wqkvT = np.ascontiguousarray(np.asarray(Wqkv, dtype=np.float32).T).astype(bf)
    woT = np.ascontiguousarray(np.asarray(Wo, dtype=np.float32).T).astype(bf)
    in_maps = []
    for c in range(NCORES):
        shard = np.ascontiguousarray(xt[:, c * T : (c + 1) * T]).astype(bf)
        in_maps.append(
            {names["xT"]: shard, names["wqkvT"]: wqkvT, names["woT"]: woT}
        )
    return in_maps


def kernel(x, Wqkv, Wo, bo, trace=False):
    in_maps = make_in_maps({"x": x, "Wqkv": Wqkv, "Wo": Wo})
    _CACHE["last_in_maps"] = in_maps
    nc, names = _CACHE["nc"], _CACHE["names"]
    res = run_bass_kernel_spmd(
        nc, in_maps, core_ids=list(range(NCORES)), trace=trace
    )
    shards = [res.results[c][names["out"]] for c in range(NCORES)]
    out = np.concatenate(shards, axis=0).reshape(B, N, D).astype(np.float32)
    out = out + np.asarray(bo, dtype=np.float32)[None, None, :]
    if trace:
        return out, res
    return out



# revision 4
# speedup vs baseline: 3.3036x; 3.3036x over previous
import sys
from contextlib import ExitStack

sys.path.insert(0, "/opt/trn_rl_repo")

import numpy as np
import ml_dtypes

import concourse.bass as bass
import concourse.bacc as bacc
import concourse.mybir as mybir
import concourse.tile as tile
from concourse.bass_utils import run_bass_kernel_spmd
from concourse.masks import make_identity

B, N, D, H, HD = 4, 4096, 1024, 16, 64
NCORES = 8
T = (B * N) // NCORES  # 2048 tokens per core
P = 128
NT = T // P            # 16 token tiles per core
KT = D // P            # 8 contraction tiles
E3 = 3 * D

# engine split knobs
N_POOL_ATTN = 8        # attn g-iterations assigned to Pool engine (rest on DVE)
N_POOL_SCORE_PACKS = 1 # of 4 four-g score-mul packs, how many go to Pool

_CACHE = {}


def _name(t):
    return t.name if hasattr(t, "name") else t.tensor.name


def _build():
    bf = mybir.dt.bfloat16
    f32 = mybir.dt.float32
    X = mybir.AxisListType.X
    nc = bacc.Bacc(None, target_bir_lowering=False)
    names = {}
    with tile.TileContext(nc) as tc:
        with ExitStack() as ctx:
            dram = ctx.enter_context(tc.tile_pool(name="dram", bufs=1, space="DRAM"))
            xT_d = dram.tile([D, T], bf, kind="ExternalInput")
            wq_d = dram.tile([D, E3], bf, kind="ExternalInput")
            wo_d = dram.tile([D, D], bf, kind="ExternalInput")
            out_d = dram.tile([T, D], f32, kind="ExternalOutput")
            names["xT"] = _name(xT_d)
            names["wqkvT"] = _name(wq_d)
            names["woT"] = _name(wo_d)
            names["out"] = _name(out_d)

            consts = ctx.enter_context(tc.tile_pool(name="consts", bufs=1))
            xT_sb = consts.tile([P, KT, T], bf)
            wq_sb = consts.tile([P, KT, E3], bf)
            wo_sb = consts.tile([P, KT, D], bf)
            ident = consts.tile([P, P], bf)
            make_identity(nc, ident)
            nc.sync.dma_start(out=xT_sb[:], in_=xT_d[:].rearrange("(k p) t -> p k t", p=P))
            nc.sync.dma_start(out=wq_sb[:], in_=wq_d[:].rearrange("(k p) e -> p k e", p=P))
            nc.sync.dma_start(out=wo_sb[:], in_=wo_d[:].rearrange("(k p) e -> p k e", p=P))

            pool = ctx.enter_context(tc.tile_pool(name="work", bufs=2))
            psum1 = ctx.enter_context(tc.tile_pool(name="psum1", bufs=2, space="PSUM"))
            psum2 = ctx.enter_context(tc.tile_pool(name="psum2", bufs=2, space="PSUM"))
            psum3 = ctx.enter_context(tc.tile_pool(name="psum3", bufs=2, space="PSUM"))

            for i in range(NT):
                tsl = bass.ts(i, P)
                # ---- QKV projection: qkv[t, e] for this 128-token tile ----
                qkv = pool.tile([P, E3], bf, tag="qkv")
                for nch in range(E3 // 512):
                    ps = psum1.tile([P, 512], f32, tag="mm1")
                    for k in range(KT):
                        nc.tensor.matmul(
                            ps[:],
                            xT_sb[:, k, tsl],
                            wq_sb[:, k, bass.ts(nch, 512)],
                            start=(k == 0),
                            stop=(k == KT - 1),
                        )
                    nc.scalar.copy(qkv[:, bass.ts(nch, 512)], ps[:])

                qv = qkv[:, 0:D].rearrange("p (h d) -> p h d", d=HD)

                # ---- scores[t, h, g] via 4-g packs: mul then strided reduce ----
                scores = pool.tile([P, H, H], f32, tag="sc")
                tmpd = pool.tile([P, 4, H, HD], bf, tag="tmpd")
                tmpp = pool.tile([P, 4, H, HD], bf, tag="tmpp")
                packs = []
                for pk in range(4):
                    on_pool = pk < N_POOL_SCORE_PACKS
                    eng = nc.gpsimd if on_pool else nc.vector
                    tmp = tmpp if on_pool else tmpd
                    k4 = qkv[:, D + pk * 4 * HD : D + (pk + 1) * 4 * HD].rearrange(
                        "p (s d) -> p s d", d=HD
                    )
                    qb = qv[:, None, :, :].broadcast_to((P, 4, H, HD))
                    kb = k4[:, :, None, :].broadcast_to((P, 4, H, HD))
                    eng.tensor_mul(tmp[:], qb, kb)
                    # reduce over d for the 4 g's of this pack (DVE only)
                    nc.vector.reduce_sum(
                        scores[:, :, pk * 4 : (pk + 1) * 4],
                        tmp[:].rearrange("p s h d -> p h s d"),
                        axis=X,
                    )
                    packs.append(tmp)

                # ---- softmax over g (no max-subtract; |s/32| is small) ----
                we = pool.tile([P, H, H], bf, tag="we")
                wen = pool.tile([P, H, H], bf, tag="wen")
                den = pool.tile([P, H], f32, tag="den")
                rec = pool.tile([P, H], f32, tag="rec")
                nc.scalar.activation(
                    we[:], scores[:], mybir.ActivationFunctionType.Exp, scale=1.0 / 32.0
                )
                nc.vector.reduce_sum(den[:], we[:], axis=X)
                nc.vector.reciprocal(rec[:], den[:])
                nc.vector.tensor_mul(
                    wen[:], we[:], rec[:, :, None].broadcast_to((P, H, H))
                )

                # ---- attn[t, h, d] = sum_g wn[t,h,g] v[t,g,d]; DVE + Pool chains ----
                attnD = pool.tile([P, H, HD], bf, tag="attnD")
                attnP = pool.tile([P, H, HD], bf, tag="attnP")
                tmgD = pool.tile([P, H, HD], bf, tag="tmgD")
                tmgP = pool.tile([P, H, HD], bf, tag="tmgP")
                n_dve = H - N_POOL_ATTN
                for g in range(H):
                    on_pool = g >= n_dve
                    eng = nc.gpsimd if on_pool else nc.vector
                    acc = attnP if on_pool else attnD
                    tmg = tmgP if on_pool else tmgD
                    vg = qkv[:, 2 * D + g * HD : 2 * D + (g + 1) * HD]
                    vgb = vg[:, None, :].broadcast_to((P, H, HD))
                    wgb = wen[:, :, g : g + 1].broadcast_to((P, H, HD))
                    first = (g == 0) or (g == n_dve)
                    if first:
                        eng.tensor_mul(acc[:], wgb, vgb)
                    else:
                        eng.tensor_mul(tmg[:], wgb, vgb)
                        eng.tensor_add(acc[:], acc[:], tmg[:])

                attnb = pool.tile([P, H, HD], bf, tag="attnb")
                nc.vector.tensor_add(attnb[:], attnD[:], attnP[:])

                # ---- transpose attn tile -> [e, t] blocks for output proj ----
                attnb_flat = attnb.rearrange("p h d -> p (h d)")
                attnT = pool.tile([P, KT, P], bf, tag="attnT")
                for c in range(KT):
                    pt = psum2.tile([P, P], bf, tag="pt")
                    nc.tensor.transpose(pt[:], attnb_flat[:, bass.ts(c, P)], ident[:])
                    nc.scalar.copy(attnT[:, c, :], pt[:])

                # ---- output projection ----
                outt = pool.tile([P, D], f32, tag="outt")
                for nch in range(D // 512):
                    po = psum3.tile([P, 512], f32, tag="po")
                    for k in range(KT):
                        nc.tensor.matmul(
                            po[:],
                            attnT[:, k, :],
                            wo_sb[:, k, bass.ts(nch, 512)],
                            start=(k == 0),
                            stop=(k == KT - 1),
                        )
                    nc.scalar.copy(outt[:, bass.ts(nch, 512)], po[:])
                nc.sync.dma_start(out=out_d[tsl, :], in_=outt[:])
    nc.compile()
    return nc, names


def make_in_maps(inputs):
    if "nc" not in _CACHE:
        _CACHE["nc"], _CACHE["names"] = _build()
    names = _CACHE["names"]
    x, Wqkv, Wo = inputs["x"], inputs["Wqkv"], inputs["Wo"]
    bf = ml_dtypes.bfloat16
    xt = np.ascontiguousarray(
        np.asarray(x, dtype=np.float32).reshape(B * N, D).T
    )  # [D, B*N]
    wqkvT = np.ascontiguousarray(np.asarray(Wqkv, dtype=np.float32).T).astype(bf)
    woT = np.ascontiguousarray(np.asarray(Wo, dtype=np.float32).T).astype(bf)
    in_maps = []
    for c in range(NCORES):
        shard = np.ascontiguousarray(xt[:, c * T : (c + 1) * T]).astype(bf)
        in_maps.append(
            {names["xT"]: shard, names["wqkvT"]: wqkvT, names["woT"]: woT}
        )
    return in_maps


def kernel(x, Wqkv, Wo, bo, trace=False):
    in_maps = make_in_maps({"x": x, "Wqkv": Wqkv, "Wo": Wo})
    _CACHE["last_in_maps"] = in_maps
    nc, names = _CACHE["nc"], _CACHE["names"]
    res = run_bass_kernel_spmd(
        nc, in_maps, core_ids=list(range(NCORES)), trace=trace
    )
    shards = [res.results[c][names["out"]] for c in range(NCORES)]
    out = np.concatenate(shards, axis=0).reshape(B, N, D).astype(np.float32)
    out = out + np.asarray(bo, dtype=np.float32)[None, None, :]
    if trace:
        return out, res
    return out
